# revision 5
# baseline (speedup 1.0000x reference)
"""Trainium2 Bass kernel for the MADE autoregressive sampling problem.

Problem: x, log_det = MADE_sample(z, W0,b0,W1,b1,W2,b2,Wout,bout)
  - 32 sequential autoregressive steps; each step runs a masked 4-layer MLP
    (32 -> 512 -> 512 -> 512 -> 64) over batch 2048 and uses only output
    column i.

Strategy (all hardcoded, self-contained):
  - Data-parallel over batch: 2048 -> 8 cores x 256. No collectives.
  - MADE degree structure: hidden unit k has degree deg(k) = k % 31. Output
    column i depends only on hidden units with degree <= i-1, which depend
    only on x[:, :i]. Sorting hidden units by degree turns every needed
    hidden slice into a prefix; each step computes only the ~17 *new* hidden
    units (degree == i-1) per layer from the prefix of the previous layer.
  - Degree groups are padded to 32-partition slots (4 groups per 128-row
    ptile, 8 ptiles) so every matmul / activation sits at a legal
    tile_position base (0/32/64/96).
  - Output-layer contributions accumulate into two persistent PSUM tiles
    (mean rows / log_stdev rows, one partition per autoregressive index);
    biases are seeded into PSUM with a K=1 matmul against a ones-row.
  - Step 0 is bias-only (mask is empty) -> x[:, 0] computed on host.
  - log_det = -sum_i log_stdev_i computed with one reduction matmul at the
    end (psum_ls rows stabilize exactly when they are read).
"""
import os
import numpy as np

import concourse.bass as bass
import concourse.mybir as mybir
import concourse.tile as tile
from concourse import bacc
from concourse.bass_utils import run_bass_kernel_spmd

D = 32        # input/output size (and number of autoregressive steps)
H = 512       # hidden size
B = 2048      # full batch
NDEG = 31     # D - 1 distinct hidden degrees
NCORES = 8
BL = B // NCORES  # per-core batch (256)
NPT = 8       # padded ptiles of 128 rows (31 groups x 32-row slots)
F32 = mybir.dt.float32
AF = mybir.ActivationFunctionType

# matmul operand dtype: float32 (4 cyc/row) or float32r (1 cyc/row at N>=256)
MM_DT = mybir.dt.float32r if os.environ.get("MADE_MM_DT", "f32") == "f32r" else F32

_CNT = np.array([17 if g < 16 else 16 for g in range(NDEG)])
_CUM = np.concatenate([[0], np.cumsum(_CNT)])  # packed offsets, _CUM[-1] == 512


def _layout():
    deg_h = np.arange(H) % NDEG
    perm = np.argsort(deg_h, kind="stable")
    pr = np.zeros(H, dtype=np.int64)  # packed idx -> padded row (0..1023)
    for g in range(NDEG):
        P, j = g // 4, g % 4
        pr[_CUM[g]:_CUM[g + 1]] = 128 * P + 32 * j + np.arange(_CNT[g])
    return deg_h, perm, pr


def _prep_weights(W0, b0, W1, b1, W2, b2, Wout, bout):
    """Host-side: masked, degree-sorted, 32-slot-padded weight layouts."""
    deg_h, perm, pr = _layout()
    deg_in = np.arange(D)
    deg_out = np.arange(D) - 1
    m0 = (deg_h[:, None] >= deg_in[None, :]).astype(np.float32)
    mh = (deg_h[:, None] >= deg_h[None, :]).astype(np.float32)
    mo = (deg_out[:, None] >= deg_h[None, :]).astype(np.float32)

    MW0 = m0 * W0
    W0s = MW0[perm]
    w0t = np.ascontiguousarray(W0s.T)  # [32, 512] K=input, cols packed out

    def wpad(W, m):
        Ws = (m * W)[perm][:, perm]
        T = np.zeros((NPT * 128, H), np.float32)
        T[pr, :] = Ws.T  # T[padded_in_row, packed_out_col]
        return np.ascontiguousarray(T.reshape(NPT, 128, H))

    w1t = wpad(W1, mh)
    w2t = wpad(W2, mh)

    def wot(Wo):  # Wo [32 out, 512 in] -> [128, NPT*32] padded-in-row blocks
        Wos = (mo * Wo)[:, perm]
        T = np.zeros((NPT * 128, D), np.float32)
        T[pr, :] = Wos.T
        out = np.zeros((128, NPT * D), np.float32)
        for P in range(NPT):
            out[:, D * P:D * P + D] = T[128 * P:128 * P + 128, :]
        return out

    womt = wot(Wout[:D])
    wolt = wot(Wout[D:])

    def bpad(b):
        bs = b[perm]
        out = np.zeros((128, NPT), np.float32)
        for g in range(NDEG):
            P, j = g // 4, g % 4
            out[32 * j:32 * j + _CNT[g], P] = bs[_CUM[g]:_CUM[g + 1]]
        return out

    return dict(
        w0t=w0t, w1t=w1t, w2t=w2t, womt=womt, wolt=wolt,
        b0p=bpad(b0), b1p=bpad(b1), b2p=bpad(b2),
        bmrow=np.ascontiguousarray(bout[:D].reshape(1, D)),
        blrow=np.ascontiguousarray(bout[D:].reshape(1, D)),
    )


def _build_nc():
    nc = bacc.Bacc("TRN2", target_bir_lowering=False, debug=False)

    def din(name, shape, dtype=F32):
        return nc.dram_tensor(name, list(shape), dtype, kind="ExternalInput").ap()

    d_zt = din("zt", [D, BL])
    d_xt0 = din("xt0", [D, BL], MM_DT)
    d_w0t = din("w0t", [D, H], MM_DT)
    d_w1t = din("w1t", [NPT, 128, H], MM_DT)
    d_w2t = din("w2t", [NPT, 128, H], MM_DT)
    d_womt = din("womt", [128, NPT * D], MM_DT)
    d_wolt = din("wolt", [128, NPT * D], MM_DT)
    d_b0p = din("b0p", [128, NPT])
    d_b1p = din("b1p", [128, NPT])
    d_b2p = din("b2p", [128, NPT])
    d_bm = din("bmrow", [1, D], MM_DT)
    d_bl = din("blrow", [1, D], MM_DT)
    d_xt_out = nc.dram_tensor("xt_out", [D, BL], F32, kind="ExternalOutput").ap()
    d_ld_out = nc.dram_tensor("ld_out", [1, BL], F32, kind="ExternalOutput").ap()

    with tile.TileContext(nc) as tc:
        with (
            tc.tile_pool(name="persist", bufs=1) as pp,
            tc.tile_pool(name="step", bufs=2) as sp,
            tc.tile_pool(name="ps_h", bufs=1, space="PSUM") as ph,
            tc.tile_pool(name="ps_acc", bufs=1, space="PSUM") as pa,
        ):
            def ptile(shape, name, dtype=F32):
                return pp.tile(shape, dtype, name=name, tag=name)

            # persistent SBUF tiles + loads
            zt = ptile([D, BL], "zt")
            nc.sync.dma_start(out=zt[:], in_=d_zt)
            xt = ptile([D, BL], "xt", dtype=MM_DT)
            nc.sync.dma_start(out=xt[:], in_=d_xt0)
            w0t = ptile([D, H], "w0t", dtype=MM_DT)
            nc.sync.dma_start(out=w0t[:], in_=d_w0t)
            w1t = [ptile([128, H], f"w1t{c}", dtype=MM_DT) for c in range(NPT)]
            w2t = [ptile([128, H], f"w2t{c}", dtype=MM_DT) for c in range(NPT)]
            for c in range(NPT):
                nc.sync.dma_start(out=w1t[c][:], in_=d_w1t[c])
                nc.sync.dma_start(out=w2t[c][:], in_=d_w2t[c])
            womt = ptile([128, NPT * D], "womt", dtype=MM_DT)
            nc.sync.dma_start(out=womt[:], in_=d_womt)
            wolt = ptile([128, NPT * D], "wolt", dtype=MM_DT)
            nc.sync.dma_start(out=wolt[:], in_=d_wolt)
            b0p = ptile([128, NPT], "b0p")
            nc.sync.dma_start(out=b0p[:], in_=d_b0p)
            b1p = ptile([128, NPT], "b1p")
            nc.sync.dma_start(out=b1p[:], in_=d_b1p)
            b2p = ptile([128, NPT], "b2p")
            nc.sync.dma_start(out=b2p[:], in_=d_b2p)
            bm = ptile([1, D], "bm", dtype=MM_DT)
            nc.sync.dma_start(out=bm[:], in_=d_bm)
            bl = ptile([1, D], "bl", dtype=MM_DT)
            nc.sync.dma_start(out=bl[:], in_=d_bl)

            ones = ptile([1, BL], "ones", dtype=MM_DT)
            nc.vector.memset(ones[:], 1.0)
            negones = ptile([D, 1], "negones", dtype=MM_DT)
            nc.vector.memset(negones[:], -1.0)

            h0 = [ptile([128, BL], f"h0_{c}", dtype=MM_DT) for c in range(NPT)]
            h1 = [ptile([128, BL], f"h1_{c}", dtype=MM_DT) for c in range(NPT)]
            for c in range(NPT):
                nc.vector.memset(h0[c][:], 0.0)
                nc.vector.memset(h1[c][:], 0.0)

            e_t = ptile([D, BL], "e_t")
            t_t = ptile([D, BL], "t_t")
            ls_sb = ptile([D, BL], "ls_sb", dtype=MM_DT)

            # persistent PSUM accumulators, seeded with output biases
            ps_mean = pa.tile([D, BL], F32, name="ps_mean", tag="ps_mean")
            ps_ls = pa.tile([D, BL], F32, name="ps_ls", tag="ps_ls")
            nc.tensor.matmul(ps_mean[:], bm[:], ones[:], start=True, stop=False,
                             skip_group_check=True)
            nc.tensor.matmul(ps_ls[:], bl[:], ones[:], start=True, stop=False,
                             skip_group_check=True)

            for i in range(1, D):
                g = i - 1
                P, j, cg = g // 4, g % 4, int(_CNT[g])
                lo, hi = int(_CUM[g]), int(_CUM[g]) + cg
                s = slice(32 * j, 32 * j + cg)
                last = i == D - 1

                ps0 = ph.tile([128, BL], F32, name="ps0", tag="ps0")
                nc.tensor.matmul(ps0[s, :], w0t[:, lo:hi], xt[:],
                                 start=True, stop=True,
                                 tile_position=(0, 32 * j))
                nc.scalar.activation(out=h0[P][s, :], in_=ps0[s, :],
                                     func=AF.Relu, bias=b0p[s, P:P + 1])

                ps1 = ph.tile([128, BL], F32, name="ps1", tag="ps1")
                for c in range(P + 1):
                    nc.tensor.matmul(ps1[s, :], w1t[c][:, lo:hi], h0[c][:],
                                     start=(c == 0), stop=(c == P),
                                     tile_position=(0, 32 * j))
                nc.scalar.activation(out=h1[P][s, :], in_=ps1[s, :],
                                     func=AF.Relu, bias=b1p[s, P:P + 1])

                ps2 = ph.tile([128, BL], F32, name="ps2", tag="ps2")
                for c in range(P + 1):
                    nc.tensor.matmul(ps2[s, :], w2t[c][:, lo:hi], h1[c][:],
                                     start=(c == 0), stop=(c == P),
                                     tile_position=(0, 32 * j))
                h2g = sp.tile([128, BL], MM_DT, name="h2g", tag="h2g")
                nc.scalar.activation(out=h2g[s, :], in_=ps2[s, :],
                                     func=AF.Relu, bias=b2p[s, P:P + 1])

                nc.tensor.matmul(ps_mean[:], womt[s, D * P:D * P + D], h2g[s, :],
                                 start=False, stop=last, skip_group_check=True,
                                 tile_position=(32 * j, 0))
                nc.tensor.matmul(ps_ls[:], wolt[s, D * P:D * P + D], h2g[s, :],
                                 start=False, stop=last, skip_group_check=True,
                                 tile_position=(32 * j, 0))

                # x[:, k] = mean_k + z_k * exp(ls_k), recomputed for ALL k
                # each step: rows < i are final (mask gives them zero updates
                # afterwards), rows > i are finite garbage that only ever
                # meets zero weights in mm0. Full-height ops keep every SBUF
                # access at start-partition 0 (HW requires 0/32/64/96) and
                # cost the same cycles as a single row (free-dim bound).
                nc.scalar.activation(out=e_t[:], in_=ps_ls[:], func=AF.Exp)
                nc.vector.tensor_mul(t_t[:], zt[:], e_t[:])
                nc.vector.tensor_add(xt[:], ps_mean[:], t_t[:])

            # log_det = -sum_i ls_i
            nc.vector.tensor_copy(ls_sb[:], ps_ls[:])
            ps_ld = pa.tile([1, BL], F32, name="ps_ld", tag="ps_ld")
            nc.tensor.matmul(ps_ld[:], negones[:], ls_sb[:], start=True, stop=True)
            ld_sb = pp.tile([1, BL], F32, name="ld_sb", tag="ld_sb")
            nc.vector.tensor_copy(ld_sb[:], ps_ld[:])

            nc.sync.dma_start(out=d_xt_out, in_=xt[:])
            nc.sync.dma_start(out=d_ld_out, in_=ld_sb[:])

    nc.compile()
    return nc


_CACHE = {}


def _get_nc():
    if "nc" not in _CACHE:
        _CACHE["nc"] = _build_nc()
    return _CACHE["nc"]


def _make_in_maps(z, W0, b0, W1, b1, W2, b2, Wout, bout):
    pd = _prep_weights(W0, b0, W1, b1, W2, b2, Wout, bout)
    in_maps = []
    for c in range(NCORES):
        sl = slice(c * BL, (c + 1) * BL)
        zt = np.ascontiguousarray(z[sl].T)
        xt0 = np.zeros((D, BL), np.float32)
        xt0[0] = bout[0] + z[sl, 0] * np.exp(bout[D])
        m = dict(pd)
        m["zt"] = zt
        m["xt0"] = xt0
        in_maps.append(m)
    return in_maps


def _assemble(results):
    x = np.zeros((B, D), np.float32)
    ld = np.zeros((B,), np.float32)
    for c in range(NCORES):
        sl = slice(c * BL, (c + 1) * BL)
        x[sl] = results[c]["xt_out"].T
        ld[sl] = results[c]["ld_out"][0]
    return x, ld


def kernel(z, W0, b0, W1, b1, W2, b2, Wout, bout):
    z = np.asarray(z, np.float32)
    args = [np.asarray(a, np.float32)
            for a in (W0, b0, W1, b1, W2, b2, Wout, bout)]
    in_maps = _make_in_maps(z, *args)
    nc = _get_nc()
    res = run_bass_kernel_spmd(nc, in_maps, list(range(NCORES))).results
    return _assemble(res)
